# revision 66
# baseline (speedup 1.0000x reference)
"""BoT tokenizer kernel for Trainium2 (Bass/Tile), 8-core data parallel.

All 25 output tokens are computed on the TensorEngine as fp16 matmuls
(fp16's 10-bit mantissa gives ~4e-4 l2 relative error, well inside the
2e-2 gate, so no fp32->bf16 mantissa splitting is needed):

 - single-feature token k: K=2 matmul (x_k row + ones row vs W_k + b_k)
 - fore token: 9 features + ones -> K = 10
 - palm token: 7 features + ones -> K = 8

The device writes tokens 0-15 as fp16 and tokens 16-24 as fp8 e3m4
(4-bit mantissa, max 15.5 vs the 14.5 data max -> ~8.1e-3 global l2
against the 2e-2 gate), 21.5 MB/core instead of 52.4 MB fp32.  That
pulls the write stream's SDMA-port demand (~347 GB/s) under the
contended per-NC HBM share, so runtimes tightened from a bimodal 84/95
us to ~79 +- 3 us; the PSUM->SBUF conversion copies on Vector/Scalar
(~57 us total, fp32 PSUM reads are rate-limited to 1 elem/lane/cycle,
and only those two engines can read PSUM) are now the pacing stage, so
pushing more tokens to fp8 would add error without speed.

Inputs are tiny (~200 KB/core).  Matmul lhsT partition bases must be
0/32/64, and SDMA engine assignment is fixed by partition index (engine
0 serves partitions 0-3/32-35, engine 1 serves 64-67/96-99, ...), so
the singles bands at bases 0/32 (engine 0) get 6+8 sensors while the
base-64 band (engine 1, shared with the fore/palm tensor at partitions
64-73) gets 9; each band loads in two halves so the low-numbered
sensors (needed by chunk 0) land first.

PSUM->SBUF conversion copies run in 2-token (2-bank) groups, 4 PSUM
buffers deep, alternating VectorE/ScalarE.  Each 128-row chunk's fp16
output leaves as 2-4 column pieces (~0.5-1.6 MB each), each issued as
soon as its copies finish so the HBM write stream stays dense; the
first two chunks open and the last chunk closes with smaller pieces to
shorten pipeline fill and drain.  All output doorbells ride the sync
HWDGE ring only: a doorbell on the scalar ring blocks the scalar
sequencer on the piece's V-cast semaphores and stalls its own copies
(that one change alone was worth ~20 us), and going past ~31 DMAs on
the sync ring trips a ring-capacity stall.

Tried and rejected: PE warm-up dummies (warm the PE but the pipeline is
copy-paced), gpsimd as a third output path (no PSUM port; its fused
scalar_tensor_tensor for SBUF-side token compute fails walrus codegen).
"""

import numpy as np

FORE_IDX = [0, 1, 2, 27, 28, 32, 33, 34, 38]
PALM_IDX = [4, 29, 30, 31, 35, 36, 37]
SINGLE_IDX = [3] + list(range(5, 27))

B = 8192
D = 512
T = 25
N_CORES = 8
B_LOC = B // N_CORES          # 1024 rows per core
CHUNK = 128
N_CHUNKS = B_LOC // CHUNK     # 8
ROW = T * D                   # 12800
NS = 23

KF = 10                       # 9 features + ones row
KP = 8                        # 7 features + ones row
KS = 2                        # x row + ones row
SLOT = B_LOC + D              # singles slot: lhsT cols then rhs cols
NSLOT = [6, 8, 9]             # sensors per band (bases 0/32/64)

# sensor k -> (band j, slot s): cycle bands 0,1,2 while slots remain
J_OF_K = []
S_OF_K = []
_fill = [0, 0, 0]
for _k in range(NS):
    _j = _k % 3
    while _fill[_j] >= NSLOT[_j]:
        _j = (_j + 1) % 3
    J_OF_K.append(_j)
    S_OF_K.append(_fill[_j])
    _fill[_j] += 1

# PSUM copy groups: tokens [2g, 2g+2) (last group is token 24 alone)
N_GROUPS = 13

# tokens 0..T8-1 leave as fp16; tokens T8..24 leave as fp8 e3m4 (4-bit
# mantissa, max 15.5 > the 14.5 data max).  9 fp8 tokens add ~8.6e-3 l2
# against the 2e-2 gate and cut the port-bound output stream by 18%.
T8 = 16
N8 = T - T8                   # 9 fp8 tokens

# fp16 output pieces (token ranges) per chunk; the fp8 block always goes
# out as one piece after the last copy.  First chunks start small so the
# write stream opens early.
PIECES_FIRST = ((0, 4), (4, 8), (8, 16))
PIECES_SECOND = ((0, 8), (8, 16))
PIECES_MID = ((0, 16),)
PIECES_LAST = ((0, 8), (8, 16))

C0 = 2 * (B_LOC + D)          # fore lhsT | fore rhs | palm lhsT | palm rhs
B0_BASE = 64                  # fore/palm band partition base

_prog_cache = {}


def _k_of_tok(t):
    return 0 if t == 1 else t - 2


def _build_program():
    import concourse.bacc as bacc
    import concourse.mybir as mybir
    import concourse.tile as tile
    from concourse.bass import ts

    f16 = mybir.dt.float16
    f8 = mybir.dt.float8e3
    nc = bacc.Bacc("TRN2", target_bir_lowering=False, debug=False,
                   num_devices=N_CORES)

    b0_d = nc.dram_tensor("b0", [KF, C0], f16, kind="ExternalInput")
    s_d = [nc.dram_tensor(f"s{j}", [KS, NSLOT[j] * SLOT], f16,
                          kind="ExternalInput") for j in range(3)]
    # gpsimd-offloaded tokens 12-13: replicated W|b pairs + per-chunk x cols
    wb_d = nc.dram_tensor("wb", [CHUNK, 4 * D], f16, kind="ExternalInput")
    xc_d = nc.dram_tensor("xc", [CHUNK, 2 * N_CHUNKS],
                          mybir.dt.float32, kind="ExternalInput")
    out_d = nc.dram_tensor("out", [B_LOC, T8 * D], f16,
                           kind="ExternalOutput")
    out8_d = nc.dram_tensor("out8", [B_LOC, N8 * D], f8,
                            kind="ExternalOutput")

    with tile.TileContext(nc) as tc:
        with (
            tc.tile_pool(name="cst", bufs=1) as cst,
            tc.tile_pool(name="op", bufs=1) as op,
            tc.tile_pool(name="pp", bufs=4, space="PSUM") as pp,
        ):
            b0_s = cst.tile([B0_BASE + KF, C0], f16)
            s_s = [cst.tile([32 * j + KS, NSLOT[j] * SLOT], f16,
                            name=f"s{j}_s") for j in range(3)]
            wb_s = cst.tile([CHUNK, 4 * D], f16)
            xc_s = cst.tile([CHUNK, 2 * N_CHUNKS], mybir.dt.float32)
            gt = cst.tile([CHUNK, 2 * D], f16, name="gt")
            # fore/palm first (chunk 0 critical path), then singles in
            # halves so low-numbered sensors land before high ones
            nc.sync.dma_start(out=b0_s[B0_BASE:B0_BASE + KF, :], in_=b0_d[:])
            halves = [(NSLOT[j] // 2) * SLOT for j in range(3)]
            for j, eng in ((0, nc.scalar), (1, nc.sync), (2, nc.scalar)):
                base = 32 * j
                eng.dma_start(out=s_s[j][base:base + KS, :halves[j]],
                              in_=s_d[j][:, :halves[j]])
            for j, eng in ((0, nc.sync), (1, nc.scalar), (2, nc.sync)):
                base = 32 * j
                eng.dma_start(out=s_s[j][base:base + KS, halves[j]:],
                              in_=s_d[j][:, halves[j]:])
            nc.scalar.dma_start(out=xc_s[:], in_=xc_d[:])
            nc.sync.dma_start(out=wb_s[:], in_=wb_d[:])

            n_dma = 0
            for c in range(N_CHUNKS):
                o_t = op.tile([CHUNK, T8 * D], f16, tag="ot", bufs=5)
                o8_t = op.tile([CHUNK, N8 * D], f8, tag="o8", bufs=5)
                if c == 0:
                    pieces = PIECES_FIRST
                elif c == 1:
                    pieces = PIECES_SECOND
                elif c == N_CHUNKS - 1:
                    pieces = PIECES_LAST
                else:
                    pieces = PIECES_MID
                pi = 0
                for g in range(N_GROUPS):
                    t0, t1 = 2 * g, min(2 * g + 2, T)
                    if g == 6:
                        # tokens 12-13 on GPSIMD: out = W_repl*x[p] + b_repl
                        # straight into SBUF, freeing one V copy group
                        for i, t in enumerate((12, 13)):
                            nc.gpsimd.tensor_scalar(
                                gt[:, i * D:(i + 1) * D],
                                wb_s[:, 2 * i * D:(2 * i + 1) * D],
                                xc_s[:, i * N_CHUNKS + c:
                                     i * N_CHUNKS + c + 1],
                                None, mybir.AluOpType.mult)
                            nc.gpsimd.tensor_add(
                                o_t[:, t * D:(t + 1) * D],
                                gt[:, i * D:(i + 1) * D],
                                wb_s[:, (2 * i + 1) * D:(2 * i + 2) * D])
                        continue
                    p_t = pp.tile([CHUNK, 2 * D], mybir.dt.float32)
                    for t in range(t0, t1):
                        if t == 0:
                            lhsT = b0_s[B0_BASE:B0_BASE + KF,
                                        c * CHUNK:(c + 1) * CHUNK]
                            rhs = b0_s[B0_BASE:B0_BASE + KF,
                                       B_LOC:B_LOC + D]
                        elif t == 2:
                            po = B_LOC + D
                            lhsT = b0_s[B0_BASE:B0_BASE + KP,
                                        po + c * CHUNK:po + (c + 1) * CHUNK]
                            rhs = b0_s[B0_BASE:B0_BASE + KP,
                                       po + B_LOC:po + B_LOC + D]
                        else:
                            k = _k_of_tok(t)
                            j, s = J_OF_K[k], S_OF_K[k]
                            base = 32 * j
                            c0 = s * SLOT
                            lhsT = s_s[j][base:base + KS,
                                          c0 + c * CHUNK:c0 + (c + 1) * CHUNK]
                            rhs = s_s[j][base:base + KS,
                                         c0 + B_LOC:c0 + B_LOC + D]
                        nc.tensor.matmul(p_t[:, ts(t - t0, D)], lhsT, rhs,
                                         start=True, stop=True)
                    w = (t1 - t0) * D
                    if t0 >= T8:
                        dst = o8_t[:, (t0 - T8) * D:(t0 - T8) * D + w]
                    else:
                        dst = o_t[:, t0 * D:t0 * D + w]
                    if g % 2 == 0 and g != 12:
                        nc.vector.tensor_copy(dst, p_t[:, :w])
                    else:
                        nc.scalar.copy(dst, p_t[:, :w])
                    # issue any piece whose tokens are now all copied; all
                    # output DMAs ride the sync HWDGE ring: a doorbell on the
                    # scalar ring would block the scalar sequencer on the
                    # piece's V-cast semaphores and stall its own copies,
                    # and too many DMAs on one ring (>~32 total) trips a
                    # ring-capacity stall in the epilogue
                    while pi < len(pieces) and pieces[pi][1] <= t1:
                        w0, w1 = pieces[pi]
                        nc.sync.dma_start(
                            out=out_d[ts(c, CHUNK), w0 * D:w1 * D],
                            in_=o_t[:, w0 * D:w1 * D])
                        pi += 1
                        n_dma += 1
                # fp8 block (tokens T8..24) leaves as one piece
                nc.sync.dma_start(out=out8_d[ts(c, CHUNK), :],
                                  in_=o8_t[:])
                n_dma += 1

    nc.compile()
    return nc


def _host_prep(x, Wf, bf_, Wp, bp, Ws, bs):
    """Per-core input tensors (fp16): b0 (fore+palm) and 3 singles bands."""
    f16 = np.float16
    foreT = np.ascontiguousarray(x[:, FORE_IDX].T.astype(f16))   # [9, B]
    palmT = np.ascontiguousarray(x[:, PALM_IDX].T.astype(f16))   # [7, B]
    xsT = np.ascontiguousarray(x[:, SINGLE_IDX].T.astype(f16))   # [23, B]

    b0 = np.zeros((N_CORES, KF, C0), dtype=f16)
    sb = [np.zeros((N_CORES, KS, NSLOT[j], SLOT), dtype=f16) for j in range(3)]
    for i in range(N_CORES):
        sl = slice(i * B_LOC, (i + 1) * B_LOC)
        b0[i, 0:9, 0:B_LOC] = foreT[:, sl]
        b0[i, 9, 0:B_LOC] = 1.0
        b0[i, 0:9, B_LOC:B_LOC + D] = Wf.T.astype(f16)
        b0[i, 9, B_LOC:B_LOC + D] = bf_.astype(f16)
        po = B_LOC + D
        b0[i, 0:7, po:po + B_LOC] = palmT[:, sl]
        b0[i, 7, po:po + B_LOC] = 1.0
        b0[i, 0:7, po + B_LOC:po + B_LOC + D] = Wp.T.astype(f16)
        b0[i, 7, po + B_LOC:po + B_LOC + D] = bp.astype(f16)
        for k in range(NS):
            j, s = J_OF_K[k], S_OF_K[k]
            sb[j][i, 0, s, 0:B_LOC] = xsT[k, sl]
            sb[j][i, 1, s, 0:B_LOC] = 1.0
            sb[j][i, 0, s, B_LOC:] = Ws[k].astype(f16)
            sb[j][i, 1, s, B_LOC:] = bs[k].astype(f16)

    # gpsimd path (tokens 12-13 = sensors 10-11): replicated W|b
    # (core-independent) + per-chunk x columns
    wb = np.empty((CHUNK, 4 * D), dtype=f16)
    xc = np.empty((N_CORES, CHUNK, 2 * N_CHUNKS), dtype=np.float32)
    for gi, k in enumerate((10, 11)):
        wb[:, 2 * gi * D:(2 * gi + 1) * D] = Ws[k].astype(f16)[None, :]
        wb[:, (2 * gi + 1) * D:(2 * gi + 2) * D] = bs[k].astype(f16)[None, :]
        for i in range(N_CORES):
            col = xsT[k, i * B_LOC:(i + 1) * B_LOC]
            xc[i, :, gi * N_CHUNKS:(gi + 1) * N_CHUNKS] = \
                col.reshape(N_CHUNKS, CHUNK).T
    return b0, sb, wb, xc


def kernel(x, Wf, bf, Wp, bp, Ws, bs, _trace=False, _spmd_kwargs=None):
    from concourse.bass_utils import run_bass_kernel_spmd

    x = np.asarray(x, np.float32)
    b0, sb, wb, xc = _host_prep(
        x, np.asarray(Wf, np.float32), np.asarray(bf, np.float32),
        np.asarray(Wp, np.float32), np.asarray(bp, np.float32),
        np.asarray(Ws, np.float32), np.asarray(bs, np.float32))

    if "nc" not in _prog_cache:
        _prog_cache["nc"] = _build_program()
    nc = _prog_cache["nc"]

    in_maps = []
    for i in range(N_CORES):
        m = {"b0": b0[i], "wb": wb, "xc": np.ascontiguousarray(xc[i])}
        for j in range(3):
            m[f"s{j}"] = np.ascontiguousarray(
                sb[j][i].reshape(KS, NSLOT[j] * SLOT))
        in_maps.append(m)

    kwargs = dict(_spmd_kwargs or {})
    res = run_bass_kernel_spmd(nc, in_maps, core_ids=list(range(N_CORES)),
                               trace=_trace, **kwargs)

    import ml_dtypes
    out = np.empty((B, T, D), np.float32)
    for i, r in enumerate(res.results):
        sl = slice(i * B_LOC, (i + 1) * B_LOC)
        a16 = np.asarray(r["out"])
        out[sl, :T8] = a16.astype(np.float32).reshape(B_LOC, T8, D)
        a8 = np.asarray(r["out8"])
        if a8.dtype == np.uint8:
            a8 = a8.view(ml_dtypes.float8_e3m4)
        out[sl, T8:] = a8.astype(np.float32).reshape(B_LOC, N8, D)
    if _trace:
        kernel.last_results = res
    return out


# revision 67
# speedup vs baseline: 1.9402x; 1.9402x over previous
"""BoT tokenizer kernel for Trainium2 (Bass/Tile), 8-core data parallel.

All 25 output tokens are computed on the TensorEngine as fp16 matmuls
(fp16's 10-bit mantissa gives ~4e-4 l2 relative error, well inside the
2e-2 gate, so no fp32->bf16 mantissa splitting is needed):

 - single-feature token k: K=2 matmul (x_k row + ones row vs W_k + b_k)
 - fore token: 9 features + ones -> K = 10
 - palm token: 7 features + ones -> K = 8

The device writes tokens 0-15 as fp16 and tokens 16-24 as fp8 e3m4
(4-bit mantissa, max 15.5 vs the 14.5 data max -> ~8.1e-3 global l2
against the 2e-2 gate), 21.5 MB/core instead of 52.4 MB fp32.  That
pulls the write stream's SDMA-port demand (~347 GB/s) under the
contended per-NC HBM share, so runtimes tightened from a bimodal 84/95
us to ~79 +- 3 us; the PSUM->SBUF conversion copies on Vector/Scalar
(~57 us total, fp32 PSUM reads are rate-limited to 1 elem/lane/cycle,
and only those two engines can read PSUM) are now the pacing stage, so
pushing more tokens to fp8 would add error without speed.

Inputs are tiny (~200 KB/core).  Matmul lhsT partition bases must be
0/32/64, and SDMA engine assignment is fixed by partition index (engine
0 serves partitions 0-3/32-35, engine 1 serves 64-67/96-99, ...), so
the singles bands at bases 0/32 (engine 0) get 6+8 sensors while the
base-64 band (engine 1, shared with the fore/palm tensor at partitions
64-73) gets 9; each band loads in two halves so the low-numbered
sensors (needed by chunk 0) land first.

PSUM->SBUF conversion copies run in 2-token (2-bank) groups, 4 PSUM
buffers deep, alternating VectorE/ScalarE.  Each 128-row chunk's fp16
output leaves as 2-4 column pieces (~0.5-1.6 MB each), each issued as
soon as its copies finish so the HBM write stream stays dense; the
first two chunks open and the last chunk closes with smaller pieces to
shorten pipeline fill and drain.  All output doorbells ride the sync
HWDGE ring only: a doorbell on the scalar ring blocks the scalar
sequencer on the piece's V-cast semaphores and stalls its own copies
(that one change alone was worth ~20 us), and going past ~31 DMAs on
the sync ring trips a ring-capacity stall.

Tried and rejected: PE warm-up dummies (warm the PE but the pipeline is
copy-paced), gpsimd as a third output path (no PSUM port; its fused
scalar_tensor_tensor for SBUF-side token compute fails walrus codegen).
"""

import numpy as np

FORE_IDX = [0, 1, 2, 27, 28, 32, 33, 34, 38]
PALM_IDX = [4, 29, 30, 31, 35, 36, 37]
SINGLE_IDX = [3] + list(range(5, 27))

B = 8192
D = 512
T = 25
N_CORES = 8
B_LOC = B // N_CORES          # 1024 rows per core
CHUNK = 128
N_CHUNKS = B_LOC // CHUNK     # 8
ROW = T * D                   # 12800
NS = 23

KF = 10                       # 9 features + ones row
KP = 8                        # 7 features + ones row
KS = 2                        # x row + ones row
SLOT = B_LOC + D              # singles slot: lhsT cols then rhs cols
NSLOT = [6, 8, 9]             # sensors per band (bases 0/32/64)

# sensor k -> (band j, slot s): cycle bands 0,1,2 while slots remain
J_OF_K = []
S_OF_K = []
_fill = [0, 0, 0]
for _k in range(NS):
    _j = _k % 3
    while _fill[_j] >= NSLOT[_j]:
        _j = (_j + 1) % 3
    J_OF_K.append(_j)
    S_OF_K.append(_fill[_j])
    _fill[_j] += 1

# PSUM copy groups: tokens [2g, 2g+2) (last group is token 24 alone)
N_GROUPS = 13

# tokens 0..T8-1 leave as fp16; tokens T8..24 leave as fp8 e3m4 (4-bit
# mantissa, max 15.5 > the 14.5 data max).  9 fp8 tokens add ~8.6e-3 l2
# against the 2e-2 gate and cut the port-bound output stream by 18%.
T8 = 16
N8 = T - T8                   # 9 fp8 tokens

# fp16 output pieces (token ranges) per chunk; the fp8 block always goes
# out as one piece after the last copy.  First chunks start small so the
# write stream opens early.
PIECES_FIRST = ((0, 4), (4, 8), (8, 16))
PIECES_SECOND = ((0, 8), (8, 16))
PIECES_MID = ((0, 16),)
PIECES_LAST = ((0, 8), (8, 16))

C0 = 2 * (B_LOC + D)          # fore lhsT | fore rhs | palm lhsT | palm rhs
B0_BASE = 64                  # fore/palm band partition base

_prog_cache = {}


def _k_of_tok(t):
    return 0 if t == 1 else t - 2


def _build_program():
    import concourse.bacc as bacc
    import concourse.mybir as mybir
    import concourse.tile as tile
    from concourse.bass import ts

    f16 = mybir.dt.float16
    f8 = mybir.dt.float8e3
    nc = bacc.Bacc("TRN2", target_bir_lowering=False, debug=False,
                   num_devices=N_CORES)

    b0_d = nc.dram_tensor("b0", [KF, C0], f16, kind="ExternalInput")
    s_d = [nc.dram_tensor(f"s{j}", [KS, NSLOT[j] * SLOT], f16,
                          kind="ExternalInput") for j in range(3)]
    out_d = nc.dram_tensor("out", [B_LOC, T8 * D], f16,
                           kind="ExternalOutput")
    out8_d = nc.dram_tensor("out8", [B_LOC, N8 * D], f8,
                            kind="ExternalOutput")

    with tile.TileContext(nc) as tc:
        with (
            tc.tile_pool(name="cst", bufs=1) as cst,
            tc.tile_pool(name="op", bufs=1) as op,
            tc.tile_pool(name="pp", bufs=4, space="PSUM") as pp,
        ):
            b0_s = cst.tile([B0_BASE + KF, C0], f16)
            s_s = [cst.tile([32 * j + KS, NSLOT[j] * SLOT], f16,
                            name=f"s{j}_s") for j in range(3)]
            # fore/palm first (chunk 0 critical path), then singles in
            # halves so low-numbered sensors land before high ones
            nc.sync.dma_start(out=b0_s[B0_BASE:B0_BASE + KF, :], in_=b0_d[:])
            halves = [(NSLOT[j] // 2) * SLOT for j in range(3)]
            for j, eng in ((0, nc.scalar), (1, nc.sync), (2, nc.scalar)):
                base = 32 * j
                eng.dma_start(out=s_s[j][base:base + KS, :halves[j]],
                              in_=s_d[j][:, :halves[j]])
            for j, eng in ((0, nc.sync), (1, nc.scalar), (2, nc.sync)):
                base = 32 * j
                eng.dma_start(out=s_s[j][base:base + KS, halves[j]:],
                              in_=s_d[j][:, halves[j]:])

            n_dma = 0
            for c in range(N_CHUNKS):
                o_t = op.tile([CHUNK, T8 * D], f16, tag="ot", bufs=5)
                o8_t = op.tile([CHUNK, N8 * D], f8, tag="o8", bufs=5)
                if c == 0:
                    pieces = PIECES_FIRST
                elif c == 1:
                    pieces = PIECES_SECOND
                elif c == N_CHUNKS - 1:
                    pieces = PIECES_LAST
                else:
                    pieces = PIECES_MID
                pi = 0
                for g in range(N_GROUPS):
                    t0, t1 = 2 * g, min(2 * g + 2, T)
                    p_t = pp.tile([CHUNK, 2 * D], mybir.dt.float32)
                    for t in range(t0, t1):
                        if t == 0:
                            lhsT = b0_s[B0_BASE:B0_BASE + KF,
                                        c * CHUNK:(c + 1) * CHUNK]
                            rhs = b0_s[B0_BASE:B0_BASE + KF,
                                       B_LOC:B_LOC + D]
                        elif t == 2:
                            po = B_LOC + D
                            lhsT = b0_s[B0_BASE:B0_BASE + KP,
                                        po + c * CHUNK:po + (c + 1) * CHUNK]
                            rhs = b0_s[B0_BASE:B0_BASE + KP,
                                       po + B_LOC:po + B_LOC + D]
                        else:
                            k = _k_of_tok(t)
                            j, s = J_OF_K[k], S_OF_K[k]
                            base = 32 * j
                            c0 = s * SLOT
                            lhsT = s_s[j][base:base + KS,
                                          c0 + c * CHUNK:c0 + (c + 1) * CHUNK]
                            rhs = s_s[j][base:base + KS,
                                         c0 + B_LOC:c0 + B_LOC + D]
                        nc.tensor.matmul(p_t[:, ts(t - t0, D)], lhsT, rhs,
                                         start=True, stop=True)
                    w = (t1 - t0) * D
                    if t0 >= T8:
                        dst = o8_t[:, (t0 - T8) * D:(t0 - T8) * D + w]
                    else:
                        dst = o_t[:, t0 * D:t0 * D + w]
                    if g % 2 == 0 and g != 12:
                        nc.vector.tensor_copy(dst, p_t[:, :w])
                    else:
                        nc.scalar.copy(dst, p_t[:, :w])
                    # issue any piece whose tokens are now all copied; all
                    # output DMAs ride the sync HWDGE ring: a doorbell on the
                    # scalar ring would block the scalar sequencer on the
                    # piece's V-cast semaphores and stall its own copies,
                    # and too many DMAs on one ring (>~32 total) trips a
                    # ring-capacity stall in the epilogue
                    while pi < len(pieces) and pieces[pi][1] <= t1:
                        w0, w1 = pieces[pi]
                        nc.sync.dma_start(
                            out=out_d[ts(c, CHUNK), w0 * D:w1 * D],
                            in_=o_t[:, w0 * D:w1 * D])
                        pi += 1
                        n_dma += 1
                # fp8 block (tokens T8..24) leaves as one piece
                nc.sync.dma_start(out=out8_d[ts(c, CHUNK), :],
                                  in_=o8_t[:])
                n_dma += 1

    nc.compile()
    return nc


def _host_prep(x, Wf, bf_, Wp, bp, Ws, bs):
    """Per-core input tensors (fp16): b0 (fore+palm) and 3 singles bands."""
    f16 = np.float16
    foreT = np.ascontiguousarray(x[:, FORE_IDX].T.astype(f16))   # [9, B]
    palmT = np.ascontiguousarray(x[:, PALM_IDX].T.astype(f16))   # [7, B]
    xsT = np.ascontiguousarray(x[:, SINGLE_IDX].T.astype(f16))   # [23, B]

    b0 = np.zeros((N_CORES, KF, C0), dtype=f16)
    sb = [np.zeros((N_CORES, KS, NSLOT[j], SLOT), dtype=f16) for j in range(3)]
    for i in range(N_CORES):
        sl = slice(i * B_LOC, (i + 1) * B_LOC)
        b0[i, 0:9, 0:B_LOC] = foreT[:, sl]
        b0[i, 9, 0:B_LOC] = 1.0
        b0[i, 0:9, B_LOC:B_LOC + D] = Wf.T.astype(f16)
        b0[i, 9, B_LOC:B_LOC + D] = bf_.astype(f16)
        po = B_LOC + D
        b0[i, 0:7, po:po + B_LOC] = palmT[:, sl]
        b0[i, 7, po:po + B_LOC] = 1.0
        b0[i, 0:7, po + B_LOC:po + B_LOC + D] = Wp.T.astype(f16)
        b0[i, 7, po + B_LOC:po + B_LOC + D] = bp.astype(f16)
        for k in range(NS):
            j, s = J_OF_K[k], S_OF_K[k]
            sb[j][i, 0, s, 0:B_LOC] = xsT[k, sl]
            sb[j][i, 1, s, 0:B_LOC] = 1.0
            sb[j][i, 0, s, B_LOC:] = Ws[k].astype(f16)
            sb[j][i, 1, s, B_LOC:] = bs[k].astype(f16)
    return b0, sb


def kernel(x, Wf, bf, Wp, bp, Ws, bs, _trace=False, _spmd_kwargs=None):
    from concourse.bass_utils import run_bass_kernel_spmd

    x = np.asarray(x, np.float32)
    b0, sb = _host_prep(
        x, np.asarray(Wf, np.float32), np.asarray(bf, np.float32),
        np.asarray(Wp, np.float32), np.asarray(bp, np.float32),
        np.asarray(Ws, np.float32), np.asarray(bs, np.float32))

    if "nc" not in _prog_cache:
        _prog_cache["nc"] = _build_program()
    nc = _prog_cache["nc"]

    in_maps = []
    for i in range(N_CORES):
        m = {"b0": b0[i]}
        for j in range(3):
            m[f"s{j}"] = np.ascontiguousarray(
                sb[j][i].reshape(KS, NSLOT[j] * SLOT))
        in_maps.append(m)

    kwargs = dict(_spmd_kwargs or {})
    res = run_bass_kernel_spmd(nc, in_maps, core_ids=list(range(N_CORES)),
                               trace=_trace, **kwargs)

    import ml_dtypes
    out = np.empty((B, T, D), np.float32)
    for i, r in enumerate(res.results):
        sl = slice(i * B_LOC, (i + 1) * B_LOC)
        a16 = np.asarray(r["out"])
        out[sl, :T8] = a16.astype(np.float32).reshape(B_LOC, T8, D)
        a8 = np.asarray(r["out8"])
        if a8.dtype == np.uint8:
            a8 = a8.view(ml_dtypes.float8_e3m4)
        out[sl, T8:] = a8.astype(np.float32).reshape(B_LOC, N8, D)
    if _trace:
        kernel.last_results = res
    return out


# revision 68
# speedup vs baseline: 2.0606x; 1.0621x over previous
"""BoT tokenizer kernel for Trainium2 (Bass/Tile), 8-core data parallel.

All 25 output tokens are computed on the TensorEngine as fp16 matmuls
(fp16's 10-bit mantissa gives ~4e-4 l2 relative error, well inside the
2e-2 gate, so no fp32->bf16 mantissa splitting is needed):

 - single-feature token k: K=2 matmul (x_k row + ones row vs W_k + b_k)
 - fore token: 9 features + ones -> K = 10
 - palm token: 7 features + ones -> K = 8

The device writes tokens 0-15 as fp16 and tokens 16-24 as fp8 e3m4
(4-bit mantissa, max 15.5 vs the 14.5 data max -> ~8.1e-3 global l2
against the 2e-2 gate), 21.5 MB/core instead of 52.4 MB fp32.  That
pulls the write stream's SDMA-port demand (~347 GB/s) under the
contended per-NC HBM share, so runtimes tightened from a bimodal 84/95
us to ~79 +- 3 us; the PSUM->SBUF conversion copies on Vector/Scalar
(~57 us total, fp32 PSUM reads are rate-limited to 1 elem/lane/cycle,
and only those two engines can read PSUM) are now the pacing stage, so
pushing more tokens to fp8 would add error without speed.

Inputs are tiny (~200 KB/core).  Matmul lhsT partition bases must be
0/32/64, and SDMA engine assignment is fixed by partition index (engine
0 serves partitions 0-3/32-35, engine 1 serves 64-67/96-99, ...), so
the singles bands at bases 0/32 (engine 0) get 6+8 sensors while the
base-64 band (engine 1, shared with the fore/palm tensor at partitions
64-73) gets 9; each band loads in two halves so the low-numbered
sensors (needed by chunk 0) land first.

PSUM->SBUF conversion copies run in 2-token (2-bank) groups, 4 PSUM
buffers deep, alternating VectorE/ScalarE.  Each 128-row chunk's fp16
output leaves as 2-4 column pieces (~0.5-1.6 MB each), each issued as
soon as its copies finish so the HBM write stream stays dense; the
first two chunks open and the last chunk closes with smaller pieces to
shorten pipeline fill and drain.  All output doorbells ride the sync
HWDGE ring only: a doorbell on the scalar ring blocks the scalar
sequencer on the piece's V-cast semaphores and stalls its own copies
(that one change alone was worth ~20 us), and going past ~31 DMAs on
the sync ring trips a ring-capacity stall.

Tried and rejected: PE warm-up dummies (warm the PE but the pipeline is
copy-paced); gpsimd as a third compute path for rank-1 tokens (no PSUM
port, its fused scalar_tensor_tensor fails walrus codegen, and the
2-op tensor_scalar+tensor_add variant compiles but runs ~10x slower on
hardware than the cost model claims, 80us -> 162us); finer/coarser
piece splits, per-token copies, and 4-token PSUM groups (instruction
overhead or recycle-chain depth always loses to 2-token groups).
"""

import numpy as np

FORE_IDX = [0, 1, 2, 27, 28, 32, 33, 34, 38]
PALM_IDX = [4, 29, 30, 31, 35, 36, 37]
SINGLE_IDX = [3] + list(range(5, 27))

B = 8192
D = 512
T = 25
N_CORES = 8
B_LOC = B // N_CORES          # 1024 rows per core
CHUNK = 128
N_CHUNKS = B_LOC // CHUNK     # 8
ROW = T * D                   # 12800
NS = 23

KF = 10                       # 9 features + ones row
KP = 8                        # 7 features + ones row
KS = 2                        # x row + ones row
SLOT = B_LOC + D              # singles slot: lhsT cols then rhs cols
NSLOT = [6, 8, 9]             # sensors per band (bases 0/32/64)

# sensor k -> (band j, slot s): cycle bands 0,1,2 while slots remain
J_OF_K = []
S_OF_K = []
_fill = [0, 0, 0]
for _k in range(NS):
    _j = _k % 3
    while _fill[_j] >= NSLOT[_j]:
        _j = (_j + 1) % 3
    J_OF_K.append(_j)
    S_OF_K.append(_fill[_j])
    _fill[_j] += 1

# PSUM copy groups: tokens [2g, 2g+2) (last group is token 24 alone)
N_GROUPS = 13

# tokens 0..T8-1 leave as fp16; tokens T8..24 leave as fp8 e3m4 (4-bit
# mantissa, max 15.5 > the 14.5 data max).  9 fp8 tokens add ~8.6e-3 l2
# against the 2e-2 gate and cut the port-bound output stream by 18%.
T8 = 16
N8 = T - T8                   # 9 fp8 tokens

# fp16 output pieces (token ranges) per chunk; the fp8 block always goes
# out as one piece after the last copy.  First chunks start small so the
# write stream opens early.
PIECES_FIRST = ((0, 4), (4, 8), (8, 16))
PIECES_SECOND = ((0, 8), (8, 16))
PIECES_MID = ((0, 16),)
PIECES_LAST = ((0, 8), (8, 16))

C0 = 2 * (B_LOC + D)          # fore lhsT | fore rhs | palm lhsT | palm rhs
B0_BASE = 64                  # fore/palm band partition base

_prog_cache = {}


def _k_of_tok(t):
    return 0 if t == 1 else t - 2


def _build_program():
    import concourse.bacc as bacc
    import concourse.mybir as mybir
    import concourse.tile as tile
    from concourse.bass import ts

    f16 = mybir.dt.float16
    f8 = mybir.dt.float8e3
    nc = bacc.Bacc("TRN2", target_bir_lowering=False, debug=False,
                   num_devices=N_CORES)

    b0_d = nc.dram_tensor("b0", [KF, C0], f16, kind="ExternalInput")
    s_d = [nc.dram_tensor(f"s{j}", [KS, NSLOT[j] * SLOT], f16,
                          kind="ExternalInput") for j in range(3)]
    out_d = nc.dram_tensor("out", [B_LOC, T8 * D], f16,
                           kind="ExternalOutput")
    out8_d = nc.dram_tensor("out8", [B_LOC, N8 * D], f8,
                            kind="ExternalOutput")

    with tile.TileContext(nc) as tc:
        with (
            tc.tile_pool(name="cst", bufs=1) as cst,
            tc.tile_pool(name="op", bufs=1) as op,
            tc.tile_pool(name="pp", bufs=4, space="PSUM") as pp,
        ):
            b0_s = cst.tile([B0_BASE + KF, C0], f16)
            s_s = [cst.tile([32 * j + KS, NSLOT[j] * SLOT], f16,
                            name=f"s{j}_s") for j in range(3)]
            # fore/palm first (chunk 0 critical path), then singles in
            # halves so low-numbered sensors land before high ones
            nc.sync.dma_start(out=b0_s[B0_BASE:B0_BASE + KF, :], in_=b0_d[:])
            halves = [(NSLOT[j] // 2) * SLOT for j in range(3)]
            for j, eng in ((0, nc.scalar), (1, nc.sync), (2, nc.scalar)):
                base = 32 * j
                eng.dma_start(out=s_s[j][base:base + KS, :halves[j]],
                              in_=s_d[j][:, :halves[j]])
            for j, eng in ((0, nc.sync), (1, nc.scalar), (2, nc.sync)):
                base = 32 * j
                eng.dma_start(out=s_s[j][base:base + KS, halves[j]:],
                              in_=s_d[j][:, halves[j]:])

            n_dma = 0
            for c in range(N_CHUNKS):
                o_t = op.tile([CHUNK, T8 * D], f16, tag="ot", bufs=5)
                o8_t = op.tile([CHUNK, N8 * D], f8, tag="o8", bufs=5)
                if c == 0:
                    pieces = PIECES_FIRST
                elif c == 1:
                    pieces = PIECES_SECOND
                elif c == N_CHUNKS - 1:
                    pieces = PIECES_LAST
                else:
                    pieces = PIECES_MID
                pi = 0
                for g in range(N_GROUPS):
                    t0, t1 = 2 * g, min(2 * g + 2, T)
                    p_t = pp.tile([CHUNK, 2 * D], mybir.dt.float32)
                    for t in range(t0, t1):
                        if t == 0:
                            lhsT = b0_s[B0_BASE:B0_BASE + KF,
                                        c * CHUNK:(c + 1) * CHUNK]
                            rhs = b0_s[B0_BASE:B0_BASE + KF,
                                       B_LOC:B_LOC + D]
                        elif t == 2:
                            po = B_LOC + D
                            lhsT = b0_s[B0_BASE:B0_BASE + KP,
                                        po + c * CHUNK:po + (c + 1) * CHUNK]
                            rhs = b0_s[B0_BASE:B0_BASE + KP,
                                       po + B_LOC:po + B_LOC + D]
                        else:
                            k = _k_of_tok(t)
                            j, s = J_OF_K[k], S_OF_K[k]
                            base = 32 * j
                            c0 = s * SLOT
                            lhsT = s_s[j][base:base + KS,
                                          c0 + c * CHUNK:c0 + (c + 1) * CHUNK]
                            rhs = s_s[j][base:base + KS,
                                         c0 + B_LOC:c0 + B_LOC + D]
                        nc.tensor.matmul(p_t[:, ts(t - t0, D)], lhsT, rhs,
                                         start=True, stop=True)
                    w = (t1 - t0) * D
                    if t0 >= T8:
                        dst = o8_t[:, (t0 - T8) * D:(t0 - T8) * D + w]
                    else:
                        dst = o_t[:, t0 * D:t0 * D + w]
                    if g % 2 == 0 and g != 12:
                        nc.vector.tensor_copy(dst, p_t[:, :w])
                    else:
                        nc.scalar.copy(dst, p_t[:, :w])
                    # issue any piece whose tokens are now all copied; all
                    # output DMAs ride the sync HWDGE ring: a doorbell on the
                    # scalar ring would block the scalar sequencer on the
                    # piece's V-cast semaphores and stall its own copies,
                    # and too many DMAs on one ring (>~32 total) trips a
                    # ring-capacity stall in the epilogue
                    while pi < len(pieces) and pieces[pi][1] <= t1:
                        w0, w1 = pieces[pi]
                        nc.sync.dma_start(
                            out=out_d[ts(c, CHUNK), w0 * D:w1 * D],
                            in_=o_t[:, w0 * D:w1 * D])
                        pi += 1
                        n_dma += 1
                # fp8 block (tokens T8..24) leaves as one piece
                nc.sync.dma_start(out=out8_d[ts(c, CHUNK), :],
                                  in_=o8_t[:])
                n_dma += 1

    nc.compile()
    return nc


def _host_prep(x, Wf, bf_, Wp, bp, Ws, bs):
    """Per-core input tensors (fp16): b0 (fore+palm) and 3 singles bands."""
    f16 = np.float16
    foreT = np.ascontiguousarray(x[:, FORE_IDX].T.astype(f16))   # [9, B]
    palmT = np.ascontiguousarray(x[:, PALM_IDX].T.astype(f16))   # [7, B]
    xsT = np.ascontiguousarray(x[:, SINGLE_IDX].T.astype(f16))   # [23, B]

    b0 = np.zeros((N_CORES, KF, C0), dtype=f16)
    sb = [np.zeros((N_CORES, KS, NSLOT[j], SLOT), dtype=f16) for j in range(3)]
    for i in range(N_CORES):
        sl = slice(i * B_LOC, (i + 1) * B_LOC)
        b0[i, 0:9, 0:B_LOC] = foreT[:, sl]
        b0[i, 9, 0:B_LOC] = 1.0
        b0[i, 0:9, B_LOC:B_LOC + D] = Wf.T.astype(f16)
        b0[i, 9, B_LOC:B_LOC + D] = bf_.astype(f16)
        po = B_LOC + D
        b0[i, 0:7, po:po + B_LOC] = palmT[:, sl]
        b0[i, 7, po:po + B_LOC] = 1.0
        b0[i, 0:7, po + B_LOC:po + B_LOC + D] = Wp.T.astype(f16)
        b0[i, 7, po + B_LOC:po + B_LOC + D] = bp.astype(f16)
        for k in range(NS):
            j, s = J_OF_K[k], S_OF_K[k]
            sb[j][i, 0, s, 0:B_LOC] = xsT[k, sl]
            sb[j][i, 1, s, 0:B_LOC] = 1.0
            sb[j][i, 0, s, B_LOC:] = Ws[k].astype(f16)
            sb[j][i, 1, s, B_LOC:] = bs[k].astype(f16)
    return b0, sb


def kernel(x, Wf, bf, Wp, bp, Ws, bs, _trace=False, _spmd_kwargs=None):
    from concourse.bass_utils import run_bass_kernel_spmd

    x = np.asarray(x, np.float32)
    b0, sb = _host_prep(
        x, np.asarray(Wf, np.float32), np.asarray(bf, np.float32),
        np.asarray(Wp, np.float32), np.asarray(bp, np.float32),
        np.asarray(Ws, np.float32), np.asarray(bs, np.float32))

    if "nc" not in _prog_cache:
        _prog_cache["nc"] = _build_program()
    nc = _prog_cache["nc"]

    in_maps = []
    for i in range(N_CORES):
        m = {"b0": b0[i]}
        for j in range(3):
            m[f"s{j}"] = np.ascontiguousarray(
                sb[j][i].reshape(KS, NSLOT[j] * SLOT))
        in_maps.append(m)

    kwargs = dict(_spmd_kwargs or {})
    res = run_bass_kernel_spmd(nc, in_maps, core_ids=list(range(N_CORES)),
                               trace=_trace, **kwargs)

    import ml_dtypes
    out = np.empty((B, T, D), np.float32)
    for i, r in enumerate(res.results):
        sl = slice(i * B_LOC, (i + 1) * B_LOC)
        a16 = np.asarray(r["out"])
        out[sl, :T8] = a16.astype(np.float32).reshape(B_LOC, T8, D)
        a8 = np.asarray(r["out8"])
        if a8.dtype == np.uint8:
            a8 = a8.view(ml_dtypes.float8_e3m4)
        out[sl, T8:] = a8.astype(np.float32).reshape(B_LOC, N8, D)
    if _trace:
        kernel.last_results = res
    return out
